# revision 1
# baseline (speedup 1.0000x reference)
"""MetaUpscale Trainium2 kernel.

Problem: x [2,64,128,128] f32, lw [256,256,576,3] f32 (per-output-pixel dynamic
weights), scale=2.  out[n, j, 2h+sh, 2w+sw] = sum_k cols[n,(h,w),k] * lw[2h+sh,2w+sw,k,j]
where cols = 3x3 unfold of x (k = ch*9 + di*3 + dj).

Strategy (memory-bound on lw, 453 MB):
- Shard H across 8 cores: core c handles source rows [16c, 16c+16) == lw rows
  [32c, 32c+32).  Per-core lw traffic 56.6 MB (28.3 MB as fp16).
- Host pre-transposes lw to W[s][j][k][q] fp16 and unfolds x to A[k][n][q] fp16
  (k on SBUF partitions in chunks of 128, q = 2048 source pixels on free dim).
- Device: DVE tensor_tensor multiply (fp16 -> 2x mode), TensorE reduces over k
  via matmul with a ones stationary vector (M=1), PSUM-accumulated over the
  5 k-chunks; ScalarE evacuates PSUM; outputs gathered and re-laid-out on host.
"""
import sys

sys.path.insert(0, "/opt/trn_rl_repo")

import numpy as np

N, C, H, W = 2, 64, 128, 128
S = 2
K = C * 9            # 576
NCORES = 8
HPC = H // NCORES    # 16 source rows per core
Q = HPC * W          # 2048 source pixels per core
KCH = [(0, 128), (128, 128), (256, 128), (384, 128), (512, 64)]

_cache = {}


def _build_nc():
    import concourse.bacc as bacc
    import concourse.tile as tile
    from concourse import mybir

    f16, f32 = mybir.dt.float16, mybir.dt.float32
    nc = bacc.Bacc("TRN2", target_bir_lowering=False, debug=False,
                   num_devices=NCORES)
    wd = nc.dram_tensor("wd", [4, 3, K, Q], f16, kind="ExternalInput")
    ad = nc.dram_tensor("ad", [K, N, Q], f16, kind="ExternalInput")
    ones_d = nc.dram_tensor("ones_d", [128, 1], f16, kind="ExternalInput")
    od = nc.dram_tensor("od", [4, 3, N, Q], f32, kind="ExternalOutput")

    with tile.TileContext(nc) as tc:
        with (
            tc.tile_pool(name="a", bufs=1) as a_pool,
            tc.tile_pool(name="w", bufs=2) as w_pool,
            tc.tile_pool(name="p", bufs=3) as p_pool,
            tc.tile_pool(name="o", bufs=3) as o_pool,
            tc.tile_pool(name="psum", bufs=8, space="PSUM") as psum_pool,
        ):
            ones_t = a_pool.tile([128, 1], f16, tag="ones")
            nc.sync.dma_start(ones_t[:], ones_d[:])

            a_sb = {}
            for kc, (ko, ks) in enumerate(KCH):
                for n in range(N):
                    t = a_pool.tile([ks, Q], f16, tag=f"a{kc}_{n}")
                    nc.sync.dma_start(t[:], ad[ko:ko + ks, n, :])
                    a_sb[kc, n] = t

            for s in range(4):
                for j in range(3):
                    w_t = []
                    for kc, (ko, ks) in enumerate(KCH):
                        t = w_pool.tile([ks, Q], f16, tag=f"w{kc}")
                        nc.sync.dma_start(t[:], wd[s, j, ko:ko + ks, :])
                        w_t.append(t)
                    out_sb = o_pool.tile([1, N * Q], f32, tag="out")
                    for n in range(N):
                        prods = []
                        for kc, (ko, ks) in enumerate(KCH):
                            p = p_pool.tile([ks, Q], f16, tag=f"p{kc}")
                            nc.vector.tensor_tensor(
                                p[:], w_t[kc][:], a_sb[kc, n][:],
                                mybir.AluOpType.mult)
                            prods.append(p)
                        for g in range(Q // 512):
                            ps = psum_pool.tile([1, 512], f32)
                            for kc, (ko, ks) in enumerate(KCH):
                                nc.tensor.matmul(
                                    ps[:], ones_t[:ks, :],
                                    prods[kc][:, g * 512:(g + 1) * 512],
                                    start=(kc == 0), stop=(kc == len(KCH) - 1))
                            nc.scalar.copy(
                                out_sb[:, n * Q + g * 512: n * Q + (g + 1) * 512],
                                ps[:])
                    nc.sync.dma_start(od[s, j].rearrange("n q -> (n q)")[None, :],
                                      out_sb[:])
    nc.compile()
    return nc


def _get_nc():
    if "nc" not in _cache:
        _cache["nc"] = _build_nc()
    return _cache["nc"]


def _prep_inputs(x, lw):
    """Build per-core in_maps (host-side shard + transpose + fp16 cast)."""
    x = np.asarray(x, dtype=np.float32)
    lw = np.asarray(lw, dtype=np.float32)

    # A[k, n, h, w]: 3x3 unfold, k = ch*9 + di*3 + dj  (torch F.unfold order)
    xp = np.pad(x, ((0, 0), (0, 0), (1, 1), (1, 1)))
    A = np.empty((C, 9, N, H, W), np.float16)
    for di in range(3):
        for dj in range(3):
            A[:, di * 3 + dj] = xp[:, :, di:di + H, dj:dj + W].transpose(1, 0, 2, 3)
    A = A.reshape(K, N, H, W)

    ones = np.ones((128, 1), np.float16)
    in_maps = []
    for c in range(NCORES):
        ad_c = np.ascontiguousarray(A[:, :, HPC * c:HPC * (c + 1), :]).reshape(K, N, Q)
        t = lw[32 * c:32 * (c + 1)].reshape(HPC, 2, W, 2, K, 3)
        # [h, sh, w, sw, k, j] -> [sh, sw, j, k, h, w]
        wd_c = t.transpose(1, 3, 5, 4, 0, 2).astype(np.float16).reshape(4, 3, K, Q)
        in_maps.append({"wd": wd_c, "ad": ad_c, "ones_d": ones})
    return in_maps


def _assemble(results):
    out = np.empty((N, 3, S * H, S * W), np.float32)
    for c in range(NCORES):
        oc = results[c]["od"]  # [4, 3, N, Q] = [(sh,sw), j, n, (h,w)]
        oc = oc.reshape(2, 2, 3, N, HPC, W)
        # -> [n, j, h, sh, w, sw]
        oc = oc.transpose(3, 2, 4, 0, 5, 1).reshape(N, 3, 2 * HPC, S * W)
        out[:, :, 32 * c:32 * (c + 1), :] = oc
    return out


def kernel(x, lw, scale):
    from concourse.bass_utils import run_bass_kernel_spmd

    nc = _get_nc()
    in_maps = _prep_inputs(x, lw)
    res = run_bass_kernel_spmd(nc, in_maps, list(range(NCORES)))
    return _assemble(res.results)
